# revision 47
# baseline (speedup 1.0000x reference)
"""Multi-head causal attention (B=4, T=2048, C=1024, H=16, DH=64) on 8 trn2 cores.

Sharding: core c owns batch b = c//2 and heads [8*(c%2), 8*(c%2)+8)  (DP over B x TP over H).

Per-core device kernel (all matmuls bf16, fp32 accumulate), software-pipelined
so ACT(exp) of head-pair j overlaps PE work of pair j+1:
  - q^T/k^T projections: head-pair-packed weights [128c, 128(2x64d)] -> one
    matmul per (pair, chunk, ct), M=128.
  - v: natural layout [t, (h d)], heads packed in N=512.
  - attention per (pair, q-chunk 512): loop causal k-tiles:
    S^T = k q^T (row-tiled pair, K=64x2, N trimmed to 512-vlo on diagonal
    tiles) -> exp on ACT (scale=1/8, masked regions skipped) -> bf16 P^T
    -> diag tril mask -> out^T[65, 512] += v_aug.T @ P^T (row 64 = rowsum).
  - normalize: bf16 PE-transposes to partition-major, reciprocal, broadcast
    multiply, write out^T bf16.
Projection/v units for the NEXT pair are interleaved between PV groups so the
PE never stalls waiting for ACT.
Host: transposes x / packs weights into SBUF-ready layouts (bf16), transposes
per-head out^T back into [B, T, H*DH] and casts to f32.
"""

import numpy as np
import ml_dtypes

B, T, C, H, DH = 4, 2048, 1024, 16, 64
NCORES = 8
HPC = H // 2  # 8 heads per core
NP = HPC // 2  # 4 head pairs per core
CT = C // 128  # 8 contraction tiles
TC = T // 512  # 4 q-chunks
TK = T // 128  # 16 k-tiles

_cache = {}


def build_program(loop_n=1, phases="123", variant=""):
    import concourse.bass as bass
    import concourse.bacc as bacc
    import concourse.mybir as mybir
    import concourse.tile as tile
    from concourse.masks import (make_upper_triangular, make_lower_triangular,
                                 make_identity)
    from contextlib import ExitStack

    f32 = mybir.dt.float32
    bf16 = mybir.dt.bfloat16
    EXP = mybir.ActivationFunctionType.Exp

    nc = bacc.Bacc("TRN2", target_bir_lowering=False, debug=False, num_devices=NCORES)
    xT_d = nc.dram_tensor("xT", [128, CT, T], bf16, kind="ExternalInput")
    wqk_d = nc.dram_tensor("wqk", [128, CT, 2, NP, 128], bf16, kind="ExternalInput")
    wv_d = nc.dram_tensor("wv", [128, CT, HPC, DH], bf16, kind="ExternalInput")
    out_d = nc.dram_tensor("out", [HPC, T, DH], bf16, kind="ExternalOutput")

    with tile.TileContext(nc) as tc, ExitStack() as ctx:
        persist = ctx.enter_context(tc.tile_pool(name="persist", bufs=1))
        ptp = ctx.enter_context(tc.tile_pool(name="ptp", bufs=6))
        normp = ctx.enter_context(tc.tile_pool(name="normp", bufs=4))
        # one shared rotation for S-score tiles AND projection accumulators
        pssp = ctx.enter_context(tc.tile_pool(name="pssp", bufs=3, space="PSUM"))
        pop = ctx.enter_context(tc.tile_pool(name="pop", bufs=2, space="PSUM"))

        # persistent SBUF
        xT = persist.tile([128, CT, T], bf16, tag="xT")
        wqk = persist.tile([128, CT, 2, NP, 128], bf16, tag="wqk")
        wv = persist.tile([128, CT, HPC, DH], bf16, tag="wv")
        qT = persist.tile([128, NP, T], bf16, tag="qT")
        kT = persist.tile([128, NP, T], bf16, tag="kT")
        vsb = persist.tile([128, TK, HPC, DH + 1], bf16, tag="vsb")
        trilT = persist.tile([128, 128], bf16, tag="trilT")

        # constants (outside the timing loop)
        make_upper_triangular(nc, trilT[:, :], val=1.0, diag=True)
        nc.gpsimd.memset(vsb[:, :, :, :], 1.0)

        def body():
            if "bigdma" in variant:
                nc.sync.dma_start(xT[:, :, :], xT_d[:, :, :])
                nc.sync.dma_start(wqk[:, :], wqk_d[:, :])
                nc.sync.dma_start(wv[:, :], wv_d[:, :])
            else:
                # input DMAs, split per contraction tile for early compute start
                for ct in range(CT):
                    nc.sync.dma_start(wqk[:, ct], wqk_d[:, ct])
                    nc.sync.dma_start(xT[:, ct, :], xT_d[:, ct, :])
                    nc.sync.dma_start(wv[:, ct], wv_d[:, ct])

            # ---- projection work units (split into halves for fine-grained
            # interleaving into the attention stream) ----
            def qk_halves(dst, dsti, j, c):
                st = {}

                def h1():
                    psw = pssp.tile([128, 1024], f32, tag="pss", name="psw")
                    ps = psw[:, 0:512]
                    st["ps"] = ps
                    for ct in range(4):
                        nc.tensor.matmul(
                            ps[:, :], wqk[:, ct, dsti, j, :],
                            xT[:, ct, bass.ts(c, 512)],
                            start=(ct == 0), stop=False,
                        )

                def h2():
                    ps = st["ps"]
                    for rep in range(2 if "w" in phases else 1):
                        for ct in range(4 if rep == 0 else 0, CT):
                            nc.tensor.matmul(
                                ps[:, :], wqk[:, ct, dsti, j, :],
                                xT[:, ct, bass.ts(c, 512)],
                                start=False, stop=(ct == CT - 1),
                            )
                    nc.vector.tensor_copy(dst[:, j, bass.ts(c, 512)], ps[:, :])

                return (h1, h2)

            def v_halves(tt):
                st = {}

                def h1():
                    psw = pssp.tile([128, 1024], f32, tag="pss", name="psw")
                    ps = psw[:, 0:512]
                    st["ps"] = ps
                    for ct in range(4):
                        nc.tensor.matmul(
                            ps[:, :], xT[:, ct, bass.ts(tt, 128)], wv[:, ct, :, :],
                            start=(ct == 0), stop=False,
                        )

                def h2():
                    ps = st["ps"]
                    for ct in range(4, CT):
                        nc.tensor.matmul(
                            ps[:, :], xT[:, ct, bass.ts(tt, 128)], wv[:, ct, :, :],
                            start=False, stop=(ct == CT - 1),
                        )
                    nc.vector.tensor_copy(
                        vsb[:, tt, :, 0:DH],
                        ps[:, :].rearrange("p (h d) -> p h d", h=HPC),
                    )

                return (h1, h2)

            def emit_qk_unit(dst, dsti, j, c):
                h1, h2 = qk_halves(dst, dsti, j, c)
                h1()
                h2()

            def emit_v_unit(tt):
                h1, h2 = v_halves(tt)
                h1()
                h2()

            if "2" not in phases:
                for j in range(NP):
                    for c in range(TC):
                        emit_qk_unit(qT, 0, j, c)
                        emit_qk_unit(kT, 1, j, c)
                for tt in range(TK):
                    emit_v_unit(tt)
                # DCE-proof consumer: write a sliver of the projections out
                nc.gpsimd.dma_start(out_d[0, 0:8, :].rearrange("a b -> (a b)"),
                                    qT[0:1, 0, 0:512])
                nc.gpsimd.dma_start(out_d[1, 0:8, :].rearrange("a b -> (a b)"),
                                    kT[0:1, 0, 0:512])
                nc.gpsimd.dma_start(out_d[2, 0:8, :].rearrange("a b -> (a b)"),
                                    vsb[0:1, 0, :, :].rearrange("p h d -> (p h d)")[0:512])
                return

            # queues of deferred work-unit halves, drained inside attention
            # chunks. vq = must-finish-this-chunk (v units); inject_q = any
            # time before the owning pair's attention starts.
            inject_q = []
            vq = []
            open_h2 = [None]  # second half of a popped unit, emitted next

            def inject(n):
                for _ in range(n):
                    if open_h2[0] is not None:
                        h2, open_h2[0] = open_h2[0], None
                        h2()
                        continue
                    src = vq if vq else inject_q
                    if not src:
                        return
                    h1, h2 = src.pop(0)
                    h1()
                    open_h2[0] = h2

            def drain_open():
                if open_h2[0] is not None:
                    h2, open_h2[0] = open_h2[0], None
                    h2()

            def drain_vq():
                drain_open()
                while vq:
                    h1, h2 = vq.pop(0)
                    h1()
                    h2()

            if "noinj" in variant:
                for j in range(NP):
                    for c in range(TC):
                        emit_qk_unit(qT, 0, j, c)
                        emit_qk_unit(kT, 1, j, c)
                for tt in range(TK):
                    emit_v_unit(tt)

            # ---- attention, pipelined across head pairs ----
            for j in range(NP):
                hA, hB = 2 * j, 2 * j + 1
                # leftovers belong to pair j: must be emitted before its attn
                drain_open()
                inject(len(inject_q))
                if j + 1 < NP and "noinj" not in variant:
                    nxt = j + 1
                    for cc in range(TC):
                        inject_q.append(qk_halves(qT, 0, nxt, cc))
                        inject_q.append(qk_halves(kT, 1, nxt, cc))

                for c in range(TC):
                    if j == 0 and "noinj" not in variant:
                        drain_open()
                        if c == 0:
                            emit_qk_unit(qT, 0, 0, 0)
                            emit_qk_unit(kT, 1, 0, 0)
                        if c + 1 < TC:
                            # next chunk's q/k ride the must-drain queue so
                            # chunk c+1 starts with its S matmuls immediately
                            vq.append(qk_halves(qT, 0, 0, c + 1))
                            vq.append(qk_halves(kT, 1, 0, c + 1))
                        vq.extend(v_halves(tt) for tt in range(4 * c, 4 * c + 4))
                    nr = 4 * c + 4
                    poA = pop.tile([128, 512], f32, tag="po")
                    poB = pop.tile([128, 512], f32, tag="po")
                    pss = {}
                    pts = {}

                    def emit_S(r):
                        diag = r >= 4 * c
                        vlo = max(0, r - 4 * c) * 128
                        ps = pssp.tile([128, 1024], f32, tag="pss")
                        pss[r] = ps
                        for rep in range(2 if "y" in phases else 1):
                            nc.tensor.matmul(
                                ps[:, vlo:512], kT[0:64, j, bass.ts(r, 128)],
                                qT[0:64, j, 512 * c + vlo:512 * (c + 1)],
                                start=True, stop=True, tile_position=(0, 0),
                            )
                            # head B shifted left by vlo so the two heads'
                            # valid regions are adjacent -> one gap-free exp
                            nc.tensor.matmul(
                                ps[:, 512:1024 - vlo], kT[64:128, j, bass.ts(r, 128)],
                                qT[64:128, j, 512 * c + vlo:512 * (c + 1)],
                                start=True, stop=True, tile_position=(64, 0),
                            )
                        pt = ptp.tile([128, 1024], bf16, tag="pt")
                        pts[r] = pt
                        for rep in range(2 if "x" in phases else 1):
                            nc.scalar.activation(pt[:, vlo:1024 - vlo],
                                                 ps[:, vlo:1024 - vlo],
                                                 EXP, scale=0.125)
                        if diag:  # diagonal tile: tril mask on DVE
                            nc.vector.tensor_mul(
                                pt[:, vlo:vlo + 128], pt[:, vlo:vlo + 128],
                                trilT[:, :])
                            nc.vector.tensor_mul(
                                pt[:, 512:640], pt[:, 512:640],
                                trilT[:, :])

                    def emit_PV(r):
                        if r >= 4 * c:
                            drain_vq()  # vsb[r] must be written before use
                        vlo = max(0, (r - 4 * c)) * 128
                        pt = pts.pop(r)
                        pss.pop(r)
                        for rep in range(2 if "z" in phases else 1):
                            # rep 1 re-accumulates: doubles out AND rowsum, so
                            # the normalized output is unchanged.
                            nc.tensor.matmul(
                                poA[0:DH + 1, vlo:512], vsb[:, r, hA, :],
                                pt[:, vlo:512],
                                start=(r == 0 and rep == 0), stop=(r == nr - 1),
                                skip_group_check=True,
                            )
                            nc.tensor.matmul(
                                poB[0:DH + 1, vlo:512], vsb[:, r, hB, :],
                                pt[:, 512:1024 - vlo],
                                start=(r == 0 and rep == 0), stop=(r == nr - 1),
                                skip_group_check=True,
                            )

                    for rr_i in range(0, nr, 2):
                        emit_S(rr_i)
                        emit_S(rr_i + 1)
                        if "i2" not in variant:
                            inject(1)
                        if rr_i >= 2:
                            emit_PV(rr_i - 2)
                            emit_PV(rr_i - 1)
                            inject(2 if "i2" in variant else 1)
                    emit_PV(nr - 2)
                    emit_PV(nr - 1)
                    inject(2 if "i2" in variant else 1)

                    # normalize + write natural-layout output rows for this chunk
                    drain_open()
                    for h, po in ((hA, poA), (hB, poB)):
                        ou_s = normp.tile([DH + 16, 512], bf16, tag="ou_s")
                        nc.vector.tensor_copy(ou_s[0:DH + 1, :], po[0:DH + 1, :])
                        # xbar transpose [80,512] -> [512,80]; row t lands at
                        # (p, x) = (t%128, t//128), i.e. t = 128x + p
                        on_T = normp.tile([128, 4, DH + 16], bf16, tag="on_T")
                        nc.sync.dma_start_transpose(on_T[:, :, :], ou_s[:, :])
                        rsc = normp.tile([128, 4], f32, tag="rsc")
                        nc.vector.reciprocal(rsc[:, :], on_T[:, :, DH])
                        on_t = normp.tile([128, 4, DH], bf16, tag="on_t")
                        for t4 in range(4):
                            nc.vector.tensor_scalar_mul(
                                on_t[:, t4, :], on_T[:, t4, 0:DH],
                                rsc[:, t4:t4 + 1])
                        nc.gpsimd.dma_start(
                            out_d[h, bass.ts(c, 512), :].rearrange(
                                "(x p) d -> p x d", p=128),
                            on_t[:, :, :])

        if loop_n > 1:
            nbody = 4 if "quad" in variant else (2 if "dbl" in variant else 1)
            with tc.For_i(0, loop_n, 1):
                for _ in range(nbody):
                    body()
        else:
            body()

    nc.compile()
    return nc


def _prep_core_inputs(x, Wq, Wk, Wv, core):
    bf = ml_dtypes.bfloat16
    b = core // 2
    hs = (core % 2) * HPC
    # x^T in SBUF layout [p, ct, t]
    xT = np.ascontiguousarray(x[b].T).astype(bf)          # [C, T]
    xT = xT.reshape(CT, 128, T).transpose(1, 0, 2)        # [128, CT, T]
    # q/k weights packed per head pair: [128, CT, 2, NP, 128]
    wqk = np.empty((128, CT, 2, NP, 128), dtype=bf)
    for dsti, W in ((0, Wq), (1, Wk)):
        for j in range(NP):
            wpair = np.concatenate(
                [W[hs + 2 * j], W[hs + 2 * j + 1]], axis=1)  # [C, 128]
            wqk[:, :, dsti, j, :] = (
                wpair.reshape(CT, 128, 128).transpose(1, 0, 2).astype(bf))
    # v weights natural: [128, CT, HPC, DH]
    wv = Wv[hs:hs + HPC].transpose(1, 0, 2)               # [C, HPC, DH]
    wv = wv.reshape(CT, 128, HPC, DH).transpose(1, 0, 2, 3).astype(bf)
    return {
        "xT": np.ascontiguousarray(xT),
        "wqk": np.ascontiguousarray(wqk),
        "wv": np.ascontiguousarray(wv),
    }


def run_on_device(inputs, loop_n=1, trace=False, phases="123"):
    """Build (cached), run on 8 cores, return per-core results."""
    from concourse.bass_utils import run_bass_kernel_spmd

    import os
    variant = os.environ.get("KVAR", "")
    key = (loop_n, phases, variant)
    if key not in _cache:
        _cache[key] = build_program(loop_n, phases, variant)
    nc = _cache[key]
    in_maps = [
        _prep_core_inputs(inputs["x"], inputs["Wq"], inputs["Wk"], inputs["Wv"], c)
        for c in range(NCORES)
    ]
    res = run_bass_kernel_spmd(nc, in_maps, list(range(NCORES)), trace=trace)
    return res


def kernel(x, Wq, Wk, Wv):
    res = run_on_device({"x": x, "Wq": Wq, "Wk": Wk, "Wv": Wv})
    out = np.empty((B, T, H * DH), np.float32)
    for core in range(NCORES):
        b = core // 2
        hs = (core % 2) * HPC
        o = np.asarray(res.results[core]["out"], dtype=np.float32)  # [HPC, T, DH]
        out[b, :, hs * DH:(hs + HPC) * DH] = o.transpose(1, 0, 2).reshape(T, HPC * DH)
    return out


# revision 48
# speedup vs baseline: 1.0151x; 1.0151x over previous
"""Multi-head causal attention (B=4, T=2048, C=1024, H=16, DH=64) on 8 trn2 cores.

Sharding: core c owns batch b = c//2 and heads [8*(c%2), 8*(c%2)+8)  (DP over B x TP over H).

Per-core device kernel (all matmuls bf16, fp32 accumulate), software-pipelined
so ACT(exp) of head-pair j overlaps PE work of pair j+1:
  - q^T/k^T projections: head-pair-packed weights [128c, 128(2x64d)] -> one
    matmul per (pair, chunk, ct), M=128.
  - v: natural layout [t, (h d)], heads packed in N=512.
  - attention per (pair, q-chunk 512): loop causal k-tiles:
    S^T = k q^T (row-tiled pair, K=64x2, N trimmed to 512-vlo on diagonal
    tiles) -> exp on ACT (scale=1/8, masked regions skipped) -> bf16 P^T
    -> diag tril mask -> out^T[65, 512] += v_aug.T @ P^T (row 64 = rowsum).
  - normalize: bf16 PE-transposes to partition-major, reciprocal, broadcast
    multiply, write out^T bf16.
Projection/v units for the NEXT pair are interleaved between PV groups so the
PE never stalls waiting for ACT.
Host: transposes x / packs weights into SBUF-ready layouts (bf16), transposes
per-head out^T back into [B, T, H*DH] and casts to f32.
"""

import numpy as np
import ml_dtypes

B, T, C, H, DH = 4, 2048, 1024, 16, 64
NCORES = 8
HPC = H // 2  # 8 heads per core
NP = HPC // 2  # 4 head pairs per core
CT = C // 128  # 8 contraction tiles
TC = T // 512  # 4 q-chunks
TK = T // 128  # 16 k-tiles

_cache = {}


def build_program(loop_n=1, phases="123", variant=""):
    import concourse.bass as bass
    import concourse.bacc as bacc
    import concourse.mybir as mybir
    import concourse.tile as tile
    from concourse.masks import (make_upper_triangular, make_lower_triangular,
                                 make_identity)
    from contextlib import ExitStack

    f32 = mybir.dt.float32
    bf16 = mybir.dt.bfloat16
    EXP = mybir.ActivationFunctionType.Exp

    nc = bacc.Bacc("TRN2", target_bir_lowering=False, debug=False, num_devices=NCORES)
    xT_d = nc.dram_tensor("xT", [128, CT, T], bf16, kind="ExternalInput")
    wqk_d = nc.dram_tensor("wqk", [128, CT, 2, NP, 128], bf16, kind="ExternalInput")
    wv_d = nc.dram_tensor("wv", [128, CT, HPC, DH], bf16, kind="ExternalInput")
    out_d = nc.dram_tensor("out", [HPC, T, DH], bf16, kind="ExternalOutput")

    with tile.TileContext(nc) as tc, ExitStack() as ctx:
        persist = ctx.enter_context(tc.tile_pool(name="persist", bufs=1))
        ptp = ctx.enter_context(tc.tile_pool(name="ptp", bufs=6))
        normp = ctx.enter_context(tc.tile_pool(name="normp", bufs=4))
        # one shared rotation for S-score tiles AND projection accumulators
        pssp = ctx.enter_context(tc.tile_pool(name="pssp", bufs=3, space="PSUM"))
        pop = ctx.enter_context(tc.tile_pool(name="pop", bufs=2, space="PSUM"))

        # persistent SBUF; inputs double-buffered so the next loop body's
        # DMAs overlap this body's compute
        xT2 = persist.tile([128, 2, CT, T], bf16, tag="xT2")
        wqk2 = persist.tile([128, 2, CT, 2, NP, 128], bf16, tag="wqk2")
        wv2 = persist.tile([128, 2, CT, HPC, DH], bf16, tag="wv2")
        qT = persist.tile([128, NP, T], bf16, tag="qT")
        kT = persist.tile([128, NP, T], bf16, tag="kT")
        vsb = persist.tile([128, TK, HPC, DH + 1], bf16, tag="vsb")
        trilT = persist.tile([128, 128], bf16, tag="trilT")

        # constants (outside the timing loop)
        make_upper_triangular(nc, trilT[:, :], val=1.0, diag=True)
        nc.gpsimd.memset(vsb[:, :, :, :], 1.0)

        def body(bi=0):
            xT = xT2[:, bi]
            wqk = wqk2[:, bi]
            wv = wv2[:, bi]
            # input DMAs, split per contraction tile for early compute start
            for ct in range(CT):
                nc.sync.dma_start(wqk[:, ct], wqk_d[:, ct])
                nc.sync.dma_start(xT[:, ct, :], xT_d[:, ct, :])
                nc.sync.dma_start(wv[:, ct], wv_d[:, ct])

            # ---- projection work units (split into halves for fine-grained
            # interleaving into the attention stream) ----
            def qk_halves(dst, dsti, j, c):
                st = {}

                def h1():
                    psw = pssp.tile([128, 1024], f32, tag="pss", name="psw")
                    ps = psw[:, 0:512]
                    st["ps"] = ps
                    for ct in range(4):
                        nc.tensor.matmul(
                            ps[:, :], wqk[:, ct, dsti, j, :],
                            xT[:, ct, bass.ts(c, 512)],
                            start=(ct == 0), stop=False,
                        )

                def h2():
                    ps = st["ps"]
                    for rep in range(2 if "w" in phases else 1):
                        for ct in range(4 if rep == 0 else 0, CT):
                            nc.tensor.matmul(
                                ps[:, :], wqk[:, ct, dsti, j, :],
                                xT[:, ct, bass.ts(c, 512)],
                                start=False, stop=(ct == CT - 1),
                            )
                    nc.vector.tensor_copy(dst[:, j, bass.ts(c, 512)], ps[:, :])

                return (h1, h2)

            def v_halves(tt):
                st = {}

                def h1():
                    psw = pssp.tile([128, 1024], f32, tag="pss", name="psw")
                    ps = psw[:, 0:512]
                    st["ps"] = ps
                    for ct in range(4):
                        nc.tensor.matmul(
                            ps[:, :], xT[:, ct, bass.ts(tt, 128)], wv[:, ct, :, :],
                            start=(ct == 0), stop=False,
                        )

                def h2():
                    ps = st["ps"]
                    for ct in range(4, CT):
                        nc.tensor.matmul(
                            ps[:, :], xT[:, ct, bass.ts(tt, 128)], wv[:, ct, :, :],
                            start=False, stop=(ct == CT - 1),
                        )
                    nc.vector.tensor_copy(
                        vsb[:, tt, :, 0:DH],
                        ps[:, :].rearrange("p (h d) -> p h d", h=HPC),
                    )

                return (h1, h2)

            def emit_qk_unit(dst, dsti, j, c):
                h1, h2 = qk_halves(dst, dsti, j, c)
                h1()
                h2()

            def emit_v_unit(tt):
                h1, h2 = v_halves(tt)
                h1()
                h2()

            if "2" not in phases:
                for j in range(NP):
                    for c in range(TC):
                        emit_qk_unit(qT, 0, j, c)
                        emit_qk_unit(kT, 1, j, c)
                for tt in range(TK):
                    emit_v_unit(tt)
                # DCE-proof consumer: write a sliver of the projections out
                nc.gpsimd.dma_start(out_d[0, 0:8, :].rearrange("a b -> (a b)"),
                                    qT[0:1, 0, 0:512])
                nc.gpsimd.dma_start(out_d[1, 0:8, :].rearrange("a b -> (a b)"),
                                    kT[0:1, 0, 0:512])
                nc.gpsimd.dma_start(out_d[2, 0:8, :].rearrange("a b -> (a b)"),
                                    vsb[0:1, 0, :, :].rearrange("p h d -> (p h d)")[0:512])
                return

            # queues of deferred work-unit halves, drained inside attention
            # chunks. vq = must-finish-this-chunk (v units); inject_q = any
            # time before the owning pair's attention starts.
            inject_q = []
            vq = []
            open_h2 = [None]  # second half of a popped unit, emitted next

            def inject(n):
                for _ in range(n):
                    if open_h2[0] is not None:
                        h2, open_h2[0] = open_h2[0], None
                        h2()
                        continue
                    src = vq if vq else inject_q
                    if not src:
                        return
                    h1, h2 = src.pop(0)
                    h1()
                    open_h2[0] = h2

            def drain_open():
                if open_h2[0] is not None:
                    h2, open_h2[0] = open_h2[0], None
                    h2()

            def drain_vq():
                drain_open()
                while vq:
                    h1, h2 = vq.pop(0)
                    h1()
                    h2()

            if "noinj" in variant:
                for j in range(NP):
                    for c in range(TC):
                        emit_qk_unit(qT, 0, j, c)
                        emit_qk_unit(kT, 1, j, c)
                for tt in range(TK):
                    emit_v_unit(tt)

            # ---- attention, pipelined across head pairs ----
            for j in range(NP):
                hA, hB = 2 * j, 2 * j + 1
                # leftovers belong to pair j: must be emitted before its attn
                drain_open()
                inject(len(inject_q))
                if j + 1 < NP and "noinj" not in variant:
                    nxt = j + 1
                    for cc in range(TC):
                        inject_q.append(qk_halves(qT, 0, nxt, cc))
                        inject_q.append(qk_halves(kT, 1, nxt, cc))

                for c in range(TC):
                    if j == 0 and "noinj" not in variant:
                        drain_open()
                        if c == 0:
                            emit_qk_unit(qT, 0, 0, 0)
                            emit_qk_unit(kT, 1, 0, 0)
                        if c + 1 < TC:
                            # next chunk's q/k ride the must-drain queue so
                            # chunk c+1 starts with its S matmuls immediately
                            vq.append(qk_halves(qT, 0, 0, c + 1))
                            vq.append(qk_halves(kT, 1, 0, c + 1))
                        vq.extend(v_halves(tt) for tt in range(4 * c, 4 * c + 4))
                    nr = 4 * c + 4
                    poA = pop.tile([128, 512], f32, tag="po")
                    poB = pop.tile([128, 512], f32, tag="po")
                    pss = {}
                    pts = {}

                    def emit_S(r):
                        diag = r >= 4 * c
                        vlo = max(0, r - 4 * c) * 128
                        ps = pssp.tile([128, 1024], f32, tag="pss")
                        pss[r] = ps
                        for rep in range(2 if "y" in phases else 1):
                            nc.tensor.matmul(
                                ps[:, vlo:512], kT[0:64, j, bass.ts(r, 128)],
                                qT[0:64, j, 512 * c + vlo:512 * (c + 1)],
                                start=True, stop=True, tile_position=(0, 0),
                            )
                            # head B shifted left by vlo so the two heads'
                            # valid regions are adjacent -> one gap-free exp
                            nc.tensor.matmul(
                                ps[:, 512:1024 - vlo], kT[64:128, j, bass.ts(r, 128)],
                                qT[64:128, j, 512 * c + vlo:512 * (c + 1)],
                                start=True, stop=True, tile_position=(64, 0),
                            )
                        pt = ptp.tile([128, 1024], bf16, tag="pt")
                        pts[r] = pt
                        for rep in range(2 if "x" in phases else 1):
                            nc.scalar.activation(pt[:, vlo:1024 - vlo],
                                                 ps[:, vlo:1024 - vlo],
                                                 EXP, scale=0.125)
                        if diag:  # diagonal tile: tril mask on DVE
                            nc.vector.tensor_mul(
                                pt[:, vlo:vlo + 128], pt[:, vlo:vlo + 128],
                                trilT[:, :])
                            nc.vector.tensor_mul(
                                pt[:, 512:640], pt[:, 512:640],
                                trilT[:, :])

                    def emit_PV(r):
                        if r >= 4 * c:
                            drain_vq()  # vsb[r] must be written before use
                        vlo = max(0, (r - 4 * c)) * 128
                        pt = pts.pop(r)
                        pss.pop(r)
                        for rep in range(2 if "z" in phases else 1):
                            # rep 1 re-accumulates: doubles out AND rowsum, so
                            # the normalized output is unchanged.
                            nc.tensor.matmul(
                                poA[0:DH + 1, vlo:512], vsb[:, r, hA, :],
                                pt[:, vlo:512],
                                start=(r == 0 and rep == 0), stop=(r == nr - 1),
                                skip_group_check=True,
                            )
                            nc.tensor.matmul(
                                poB[0:DH + 1, vlo:512], vsb[:, r, hB, :],
                                pt[:, 512:1024 - vlo],
                                start=(r == 0 and rep == 0), stop=(r == nr - 1),
                                skip_group_check=True,
                            )

                    for rr_i in range(0, nr, 2):
                        emit_S(rr_i)
                        emit_S(rr_i + 1)
                        if "i2" not in variant:
                            inject(1)
                        if rr_i >= 2:
                            emit_PV(rr_i - 2)
                            emit_PV(rr_i - 1)
                            inject(2 if "i2" in variant else 1)
                    emit_PV(nr - 2)
                    emit_PV(nr - 1)
                    inject(2 if "i2" in variant else 1)

                    # normalize + write natural-layout output rows for this chunk
                    drain_open()
                    for h, po in ((hA, poA), (hB, poB)):
                        ou_s = normp.tile([DH + 16, 512], bf16, tag="ou_s")
                        nc.vector.tensor_copy(ou_s[0:DH + 1, :], po[0:DH + 1, :])
                        # xbar transpose [80,512] -> [512,80]; row t lands at
                        # (p, x) = (t%128, t//128), i.e. t = 128x + p
                        on_T = normp.tile([128, 4, DH + 16], bf16, tag="on_T")
                        nc.sync.dma_start_transpose(on_T[:, :, :], ou_s[:, :])
                        rsc = normp.tile([128, 4], f32, tag="rsc")
                        nc.vector.reciprocal(rsc[:, :], on_T[:, :, DH])
                        on_t = normp.tile([128, 4, DH], bf16, tag="on_t")
                        for t4 in range(4):
                            nc.vector.tensor_scalar_mul(
                                on_t[:, t4, :], on_T[:, t4, 0:DH],
                                rsc[:, t4:t4 + 1])
                        nc.gpsimd.dma_start(
                            out_d[h, bass.ts(c, 512), :].rearrange(
                                "(x p) d -> p x d", p=128),
                            on_t[:, :, :])

        if loop_n > 1:
            nbody = 4 if "quad" in variant else (2 if "dbl" in variant else 1)
            with tc.For_i(0, loop_n, 1):
                for b_i in range(nbody):
                    body(b_i % 2)
        else:
            body()

    nc.compile()
    return nc


def _prep_core_inputs(x, Wq, Wk, Wv, core):
    bf = ml_dtypes.bfloat16
    b = core // 2
    hs = (core % 2) * HPC
    # x^T in SBUF layout [p, ct, t]
    xT = np.ascontiguousarray(x[b].T).astype(bf)          # [C, T]
    xT = xT.reshape(CT, 128, T).transpose(1, 0, 2)        # [128, CT, T]
    # q/k weights packed per head pair: [128, CT, 2, NP, 128]
    wqk = np.empty((128, CT, 2, NP, 128), dtype=bf)
    for dsti, W in ((0, Wq), (1, Wk)):
        for j in range(NP):
            wpair = np.concatenate(
                [W[hs + 2 * j], W[hs + 2 * j + 1]], axis=1)  # [C, 128]
            wqk[:, :, dsti, j, :] = (
                wpair.reshape(CT, 128, 128).transpose(1, 0, 2).astype(bf))
    # v weights natural: [128, CT, HPC, DH]
    wv = Wv[hs:hs + HPC].transpose(1, 0, 2)               # [C, HPC, DH]
    wv = wv.reshape(CT, 128, HPC, DH).transpose(1, 0, 2, 3).astype(bf)
    return {
        "xT": np.ascontiguousarray(xT),
        "wqk": np.ascontiguousarray(wqk),
        "wv": np.ascontiguousarray(wv),
    }


def run_on_device(inputs, loop_n=1, trace=False, phases="123"):
    """Build (cached), run on 8 cores, return per-core results."""
    from concourse.bass_utils import run_bass_kernel_spmd

    import os
    variant = os.environ.get("KVAR", "")
    key = (loop_n, phases, variant)
    if key not in _cache:
        _cache[key] = build_program(loop_n, phases, variant)
    nc = _cache[key]
    in_maps = [
        _prep_core_inputs(inputs["x"], inputs["Wq"], inputs["Wk"], inputs["Wv"], c)
        for c in range(NCORES)
    ]
    res = run_bass_kernel_spmd(nc, in_maps, list(range(NCORES)), trace=trace)
    return res


def kernel(x, Wq, Wk, Wv):
    res = run_on_device({"x": x, "Wq": Wq, "Wk": Wk, "Wv": Wv})
    out = np.empty((B, T, H * DH), np.float32)
    for core in range(NCORES):
        b = core // 2
        hs = (core % 2) * HPC
        o = np.asarray(res.results[core]["out"], dtype=np.float32)  # [HPC, T, DH]
        out[b, :, hs * DH:(hs + HPC) * DH] = o.transpose(1, 0, 2).reshape(T, HPC * DH)
    return out


# revision 50
# speedup vs baseline: 1.0372x; 1.0217x over previous
"""Multi-head causal attention (B=4, T=2048, C=1024, H=16, DH=64) on 8 trn2 cores.

Sharding: core c owns batch b = c//2 and heads [8*(c%2), 8*(c%2)+8)  (DP over B x TP over H).

Per-core device kernel (all matmuls bf16, fp32 accumulate), software-pipelined
so ACT(exp) of head-pair j overlaps PE work of pair j+1:
  - q^T/k^T projections: head-pair-packed weights [128c, 128(2x64d)] -> one
    matmul per (pair, chunk, ct), M=128.
  - v: natural layout [t, (h d)], heads packed in N=512.
  - attention per (pair, q-chunk 512): loop causal k-tiles:
    S^T = k q^T (row-tiled pair, K=64x2, N trimmed to 512-vlo on diagonal
    tiles) -> exp on ACT (scale=1/8, masked regions skipped) -> bf16 P^T
    -> diag tril mask -> out^T[65, 512] += v_aug.T @ P^T (row 64 = rowsum).
  - normalize: bf16 PE-transposes to partition-major, reciprocal, broadcast
    multiply, write out^T bf16.
Projection/v units for the NEXT pair are interleaved between PV groups so the
PE never stalls waiting for ACT.
Host: transposes x / packs weights into SBUF-ready layouts (bf16), transposes
per-head out^T back into [B, T, H*DH] and casts to f32.
"""

import numpy as np
import ml_dtypes

B, T, C, H, DH = 4, 2048, 1024, 16, 64
NCORES = 8
HPC = H // 2  # 8 heads per core
NP = HPC // 2  # 4 head pairs per core
CT = C // 128  # 8 contraction tiles
TC = T // 512  # 4 q-chunks
TK = T // 128  # 16 k-tiles

_cache = {}


def build_program(loop_n=1, phases="123", variant=""):
    import concourse.bass as bass
    import concourse.bacc as bacc
    import concourse.mybir as mybir
    import concourse.tile as tile
    from concourse.masks import (make_upper_triangular, make_lower_triangular,
                                 make_identity)
    from contextlib import ExitStack

    f32 = mybir.dt.float32
    bf16 = mybir.dt.bfloat16
    EXP = mybir.ActivationFunctionType.Exp

    nc = bacc.Bacc("TRN2", target_bir_lowering=False, debug=False, num_devices=NCORES)
    xT_d = nc.dram_tensor("xT", [128, CT, T], bf16, kind="ExternalInput")
    wqk_d = nc.dram_tensor("wqk", [128, CT, 2, NP, 128], bf16, kind="ExternalInput")
    wv_d = nc.dram_tensor("wv", [128, CT, HPC, DH], bf16, kind="ExternalInput")
    out_d = nc.dram_tensor("out", [HPC, T, DH], bf16, kind="ExternalOutput")

    with tile.TileContext(nc) as tc, ExitStack() as ctx:
        persist = ctx.enter_context(tc.tile_pool(name="persist", bufs=1))
        ptp = ctx.enter_context(tc.tile_pool(name="ptp", bufs=6))
        normp = ctx.enter_context(tc.tile_pool(name="normp", bufs=4))
        # one shared rotation for S-score tiles AND projection accumulators
        pssp = ctx.enter_context(tc.tile_pool(name="pssp", bufs=3, space="PSUM"))
        pop = ctx.enter_context(tc.tile_pool(name="pop", bufs=2, space="PSUM"))

        # persistent SBUF; inputs double-buffered so the next loop body's
        # DMAs overlap this body's compute
        xT2 = persist.tile([128, 2, CT, T], bf16, tag="xT2")
        wqk2 = persist.tile([128, 2, CT, 2, NP, 128], bf16, tag="wqk2")
        wv2 = persist.tile([128, 2, CT, HPC, DH], bf16, tag="wv2")
        qT = persist.tile([128, NP, T], bf16, tag="qT")
        kT = persist.tile([128, NP, T], bf16, tag="kT")
        vsb = persist.tile([128, TK, HPC, DH + 1], bf16, tag="vsb")
        trilT = persist.tile([128, 128], bf16, tag="trilT")

        # constants (outside the timing loop)
        make_upper_triangular(nc, trilT[:, :], val=1.0, diag=True)
        nc.gpsimd.memset(vsb[:, :, :, :], 1.0)

        def mk_qk_halves(xT, wqk, dst, dsti, j, c):
            st = {}

            def h1():
                psw = pssp.tile([128, 1024], f32, tag="pss", name="psw")
                ps = psw[:, 0:512]
                st["ps"] = ps
                for ct in range(4):
                    nc.tensor.matmul(
                        ps[:, :], wqk[:, ct, dsti, j, :],
                        xT[:, ct, bass.ts(c, 512)],
                        start=(ct == 0), stop=False,
                    )

            def h2():
                ps = st["ps"]
                for rep in range(2 if "w" in phases else 1):
                    for ct in range(4 if rep == 0 else 0, CT):
                        nc.tensor.matmul(
                            ps[:, :], wqk[:, ct, dsti, j, :],
                            xT[:, ct, bass.ts(c, 512)],
                            start=False, stop=(ct == CT - 1),
                        )
                nc.vector.tensor_copy(dst[:, j, bass.ts(c, 512)], ps[:, :])

            return (h1, h2)

        def emit_dmas(b):
            # input DMAs, split per contraction tile for early compute start
            for ct in range(CT):
                nc.sync.dma_start(wqk2[:, b, ct], wqk_d[:, ct])
                nc.sync.dma_start(xT2[:, b, ct, :], xT_d[:, ct, :])
                nc.sync.dma_start(wv2[:, b, ct], wv_d[:, ct])

        def body(bi=0, dma_bis=(0,), carry_in=None, pair0_done=False):
            xT = xT2[:, bi]
            wqk = wqk2[:, bi]
            wv = wv2[:, bi]
            for b in dma_bis:
                emit_dmas(b)

            # ---- projection work units (split into halves for fine-grained
            # interleaving into the attention stream) ----
            def qk_halves(dst, dsti, j, c):
                return mk_qk_halves(xT, wqk, dst, dsti, j, c)

            def v_halves(tt):
                st = {}

                def h1():
                    psw = pssp.tile([128, 1024], f32, tag="pss", name="psw")
                    ps = psw[:, 0:512]
                    st["ps"] = ps
                    for ct in range(4):
                        nc.tensor.matmul(
                            ps[:, :], xT[:, ct, bass.ts(tt, 128)], wv[:, ct, :, :],
                            start=(ct == 0), stop=False,
                        )

                def h2():
                    ps = st["ps"]
                    for ct in range(4, CT):
                        nc.tensor.matmul(
                            ps[:, :], xT[:, ct, bass.ts(tt, 128)], wv[:, ct, :, :],
                            start=False, stop=(ct == CT - 1),
                        )
                    nc.vector.tensor_copy(
                        vsb[:, tt, :, 0:DH],
                        ps[:, :].rearrange("p (h d) -> p h d", h=HPC),
                    )

                return (h1, h2)

            def emit_qk_unit(dst, dsti, j, c):
                h1, h2 = qk_halves(dst, dsti, j, c)
                h1()
                h2()

            def emit_v_unit(tt):
                h1, h2 = v_halves(tt)
                h1()
                h2()

            if "2" not in phases:
                for j in range(NP):
                    for c in range(TC):
                        emit_qk_unit(qT, 0, j, c)
                        emit_qk_unit(kT, 1, j, c)
                for tt in range(TK):
                    emit_v_unit(tt)
                # DCE-proof consumer: write a sliver of the projections out
                nc.gpsimd.dma_start(out_d[0, 0:8, :].rearrange("a b -> (a b)"),
                                    qT[0:1, 0, 0:512])
                nc.gpsimd.dma_start(out_d[1, 0:8, :].rearrange("a b -> (a b)"),
                                    kT[0:1, 0, 0:512])
                nc.gpsimd.dma_start(out_d[2, 0:8, :].rearrange("a b -> (a b)"),
                                    vsb[0:1, 0, :, :].rearrange("p h d -> (p h d)")[0:512])
                return

            # queues of deferred work-unit halves, drained inside attention
            # chunks. vq = must-finish-this-chunk (v units); inject_q = any
            # time before the owning pair's attention starts.
            inject_q = []
            vq = []
            open_h2 = [None]  # second half of a popped unit, emitted next

            def inject(n):
                for _ in range(n):
                    if open_h2[0] is not None:
                        h2, open_h2[0] = open_h2[0], None
                        h2()
                        continue
                    src = vq if vq else inject_q
                    if not src:
                        return
                    h1, h2 = src.pop(0)
                    h1()
                    open_h2[0] = h2

            def drain_open():
                if open_h2[0] is not None:
                    h2, open_h2[0] = open_h2[0], None
                    h2()

            def drain_vq():
                drain_open()
                while vq:
                    h1, h2 = vq.pop(0)
                    h1()
                    h2()

            if "noinj" in variant:
                for j in range(NP):
                    for c in range(TC):
                        emit_qk_unit(qT, 0, j, c)
                        emit_qk_unit(kT, 1, j, c)
                for tt in range(TK):
                    emit_v_unit(tt)

            # ---- attention, pipelined across head pairs ----
            for j in range(NP):
                hA, hB = 2 * j, 2 * j + 1
                # leftovers belong to pair j: must be emitted before its attn
                drain_open()
                inject(len(inject_q))
                if j + 1 < NP and "noinj" not in variant:
                    nxt = j + 1
                    for cc in range(TC):
                        inject_q.append(qk_halves(qT, 0, nxt, cc))
                        inject_q.append(qk_halves(kT, 1, nxt, cc))
                elif j + 1 == NP and carry_in:
                    # next body's pair-0 projections fill this ACT-paced pair
                    inject_q.extend(carry_in)

                for c in range(TC):
                    if j == 0 and "noinj" not in variant:
                        drain_open()
                        if c == 0 and not pair0_done:
                            emit_qk_unit(qT, 0, 0, 0)
                            emit_qk_unit(kT, 1, 0, 0)
                        if c + 1 < TC and not pair0_done:
                            # next chunk's q/k ride the must-drain queue so
                            # chunk c+1 starts with its S matmuls immediately
                            vq.append(qk_halves(qT, 0, 0, c + 1))
                            vq.append(qk_halves(kT, 1, 0, c + 1))
                        vq.extend(v_halves(tt) for tt in range(4 * c, 4 * c + 4))
                    nr = 4 * c + 4
                    poA = pop.tile([128, 512], f32, tag="po")
                    poB = pop.tile([128, 512], f32, tag="po")
                    pss = {}
                    pts = {}

                    def emit_S(r):
                        diag = r >= 4 * c
                        vlo = max(0, r - 4 * c) * 128
                        ps = pssp.tile([128, 1024], f32, tag="pss")
                        pss[r] = ps
                        for rep in range(2 if "y" in phases else 1):
                            nc.tensor.matmul(
                                ps[:, vlo:512], kT[0:64, j, bass.ts(r, 128)],
                                qT[0:64, j, 512 * c + vlo:512 * (c + 1)],
                                start=True, stop=True, tile_position=(0, 0),
                            )
                            # head B shifted left by vlo so the two heads'
                            # valid regions are adjacent -> one gap-free exp
                            nc.tensor.matmul(
                                ps[:, 512:1024 - vlo], kT[64:128, j, bass.ts(r, 128)],
                                qT[64:128, j, 512 * c + vlo:512 * (c + 1)],
                                start=True, stop=True, tile_position=(64, 0),
                            )
                        pt = ptp.tile([128, 1024], bf16, tag="pt")
                        pts[r] = pt
                        for rep in range(2 if "x" in phases else 1):
                            nc.scalar.activation(pt[:, vlo:1024 - vlo],
                                                 ps[:, vlo:1024 - vlo],
                                                 EXP, scale=0.125)
                        if diag:  # diagonal tile: tril mask on DVE
                            nc.vector.tensor_mul(
                                pt[:, vlo:vlo + 128], pt[:, vlo:vlo + 128],
                                trilT[:, :])
                            nc.vector.tensor_mul(
                                pt[:, 512:640], pt[:, 512:640],
                                trilT[:, :])

                    def emit_PV(r):
                        if r >= 4 * c:
                            drain_vq()  # vsb[r] must be written before use
                        vlo = max(0, (r - 4 * c)) * 128
                        pt = pts.pop(r)
                        pss.pop(r)
                        for rep in range(2 if "z" in phases else 1):
                            # rep 1 re-accumulates: doubles out AND rowsum, so
                            # the normalized output is unchanged.
                            nc.tensor.matmul(
                                poA[0:DH + 1, vlo:512], vsb[:, r, hA, :],
                                pt[:, vlo:512],
                                start=(r == 0 and rep == 0), stop=(r == nr - 1),
                                skip_group_check=True,
                            )
                            nc.tensor.matmul(
                                poB[0:DH + 1, vlo:512], vsb[:, r, hB, :],
                                pt[:, 512:1024 - vlo],
                                start=(r == 0 and rep == 0), stop=(r == nr - 1),
                                skip_group_check=True,
                            )

                    for rr_i in range(0, nr, 2):
                        emit_S(rr_i)
                        emit_S(rr_i + 1)
                        if "i2" not in variant:
                            inject(1)
                        if rr_i >= 2:
                            emit_PV(rr_i - 2)
                            emit_PV(rr_i - 1)
                            inject(2 if "i2" in variant else 1)
                    emit_PV(nr - 2)
                    emit_PV(nr - 1)
                    inject(2 if "i2" in variant else 1)

                    # normalize + write natural-layout output rows for this chunk
                    drain_open()
                    for h, po in ((hA, poA), (hB, poB)):
                        ou_s = normp.tile([DH + 16, 512], bf16, tag="ou_s")
                        nc.vector.tensor_copy(ou_s[0:DH + 1, :], po[0:DH + 1, :])
                        # xbar transpose [80,512] -> [512,80]; row t lands at
                        # (p, x) = (t%128, t//128), i.e. t = 128x + p
                        on_T = normp.tile([128, 4, DH + 16], bf16, tag="on_T")
                        nc.sync.dma_start_transpose(on_T[:, :, :], ou_s[:, :])
                        rsc = normp.tile([128, 4], f32, tag="rsc")
                        nc.vector.reciprocal(rsc[:, :], on_T[:, :, DH])
                        on_t = normp.tile([128, 4, DH], bf16, tag="on_t")
                        for t4 in range(4):
                            nc.vector.tensor_scalar_mul(
                                on_t[:, t4, :], on_T[:, t4, 0:DH],
                                rsc[:, t4:t4 + 1])
                        nc.gpsimd.dma_start(
                            out_d[h, bass.ts(c, 512), :].rearrange(
                                "(x p) d -> p x d", p=128),
                            on_t[:, :, :])

            # all queued work (incl. next body's carry) must emit in this body
            drain_open()
            inject(len(inject_q))

        if loop_n > 1:
            nbody = 4 if "quad" in variant else (2 if "dbl" in variant else 1)
            with tc.For_i(0, loop_n, 1):
                for b_i in range(nbody):
                    bi = b_i % 2
                    if b_i + 1 < nbody:
                        nxt = (b_i + 1) % 2
                        carry = []
                        for cc in range(TC):
                            carry.append(mk_qk_halves(
                                xT2[:, nxt], wqk2[:, nxt], qT, 0, 0, cc))
                            carry.append(mk_qk_halves(
                                xT2[:, nxt], wqk2[:, nxt], kT, 1, 0, cc))
                    else:
                        carry = None
                    body(bi, dma_bis=((0, 1) if b_i == 0 else ()),
                         carry_in=carry, pair0_done=(b_i > 0))
        else:
            body()

    nc.compile()
    return nc


def _prep_core_inputs(x, Wq, Wk, Wv, core):
    bf = ml_dtypes.bfloat16
    b = core // 2
    hs = (core % 2) * HPC
    # x^T in SBUF layout [p, ct, t]
    xT = np.ascontiguousarray(x[b].T).astype(bf)          # [C, T]
    xT = xT.reshape(CT, 128, T).transpose(1, 0, 2)        # [128, CT, T]
    # q/k weights packed per head pair: [128, CT, 2, NP, 128]
    wqk = np.empty((128, CT, 2, NP, 128), dtype=bf)
    for dsti, W in ((0, Wq), (1, Wk)):
        for j in range(NP):
            wpair = np.concatenate(
                [W[hs + 2 * j], W[hs + 2 * j + 1]], axis=1)  # [C, 128]
            wqk[:, :, dsti, j, :] = (
                wpair.reshape(CT, 128, 128).transpose(1, 0, 2).astype(bf))
    # v weights natural: [128, CT, HPC, DH]
    wv = Wv[hs:hs + HPC].transpose(1, 0, 2)               # [C, HPC, DH]
    wv = wv.reshape(CT, 128, HPC, DH).transpose(1, 0, 2, 3).astype(bf)
    return {
        "xT": np.ascontiguousarray(xT),
        "wqk": np.ascontiguousarray(wqk),
        "wv": np.ascontiguousarray(wv),
    }


def run_on_device(inputs, loop_n=1, trace=False, phases="123"):
    """Build (cached), run on 8 cores, return per-core results."""
    from concourse.bass_utils import run_bass_kernel_spmd

    import os
    variant = os.environ.get("KVAR", "")
    key = (loop_n, phases, variant)
    if key not in _cache:
        _cache[key] = build_program(loop_n, phases, variant)
    nc = _cache[key]
    in_maps = [
        _prep_core_inputs(inputs["x"], inputs["Wq"], inputs["Wk"], inputs["Wv"], c)
        for c in range(NCORES)
    ]
    res = run_bass_kernel_spmd(nc, in_maps, list(range(NCORES)), trace=trace)
    return res


def kernel(x, Wq, Wk, Wv):
    res = run_on_device({"x": x, "Wq": Wq, "Wk": Wk, "Wv": Wv})
    out = np.empty((B, T, H * DH), np.float32)
    for core in range(NCORES):
        b = core // 2
        hs = (core % 2) * HPC
        o = np.asarray(res.results[core]["out"], dtype=np.float32)  # [HPC, T, DH]
        out[b, :, hs * DH:(hs + HPC) * DH] = o.transpose(1, 0, 2).reshape(T, HPC * DH)
    return out


# revision 51
# speedup vs baseline: 1.0385x; 1.0013x over previous
"""Multi-head causal attention (B=4, T=2048, C=1024, H=16, DH=64) on 8 trn2 cores.

Sharding: core c owns batch b = c//2 and heads [8*(c%2), 8*(c%2)+8)  (DP over B x TP over H).

Per-core device kernel (all matmuls bf16, fp32 accumulate), software-pipelined
so ACT(exp) of head-pair j overlaps PE work of pair j+1:
  - q^T/k^T projections: head-pair-packed weights [128c, 128(2x64d)] -> one
    matmul per (pair, chunk, ct), M=128.
  - v: natural layout [t, (h d)], heads packed in N=512.
  - attention per (pair, q-chunk 512): loop causal k-tiles:
    S^T = k q^T (row-tiled pair, K=64x2, N trimmed to 512-vlo on diagonal
    tiles) -> exp on ACT (scale=1/8, masked regions skipped) -> bf16 P^T
    -> diag tril mask -> out^T[65, 512] += v_aug.T @ P^T (row 64 = rowsum).
  - normalize: bf16 PE-transposes to partition-major, reciprocal, broadcast
    multiply, write out^T bf16.
Projection/v units for the NEXT pair are interleaved between PV groups so the
PE never stalls waiting for ACT.
Host: transposes x / packs weights into SBUF-ready layouts (bf16), transposes
per-head out^T back into [B, T, H*DH] and casts to f32.
"""

import numpy as np
import ml_dtypes

B, T, C, H, DH = 4, 2048, 1024, 16, 64
NCORES = 8
HPC = H // 2  # 8 heads per core
NP = HPC // 2  # 4 head pairs per core
CT = C // 128  # 8 contraction tiles
TC = T // 512  # 4 q-chunks
TK = T // 128  # 16 k-tiles

_cache = {}


def build_program(loop_n=1, phases="123", variant=""):
    import concourse.bass as bass
    import concourse.bacc as bacc
    import concourse.mybir as mybir
    import concourse.tile as tile
    from concourse.masks import (make_upper_triangular, make_lower_triangular,
                                 make_identity)
    from contextlib import ExitStack

    f32 = mybir.dt.float32
    bf16 = mybir.dt.bfloat16
    EXP = mybir.ActivationFunctionType.Exp

    nc = bacc.Bacc("TRN2", target_bir_lowering=False, debug=False, num_devices=NCORES)
    xT_d = nc.dram_tensor("xT", [128, CT, T], bf16, kind="ExternalInput")
    wqk_d = nc.dram_tensor("wqk", [128, CT, 2, NP, 128], bf16, kind="ExternalInput")
    wv_d = nc.dram_tensor("wv", [128, CT, HPC, DH], bf16, kind="ExternalInput")
    out_d = nc.dram_tensor("out", [HPC, T, DH], bf16, kind="ExternalOutput")

    with tile.TileContext(nc) as tc, ExitStack() as ctx:
        persist = ctx.enter_context(tc.tile_pool(name="persist", bufs=1))
        ptp = ctx.enter_context(tc.tile_pool(name="ptp", bufs=6))
        normp = ctx.enter_context(tc.tile_pool(name="normp", bufs=4))
        # one shared rotation for S-score tiles AND projection accumulators
        pssp = ctx.enter_context(tc.tile_pool(name="pssp", bufs=3, space="PSUM"))
        pop = ctx.enter_context(tc.tile_pool(name="pop", bufs=2, space="PSUM"))

        # persistent SBUF; inputs double-buffered so the next loop body's
        # DMAs overlap this body's compute
        xT2 = persist.tile([128, 2, CT, T], bf16, tag="xT2")
        wqk2 = persist.tile([128, 2, CT, 2, NP, 128], bf16, tag="wqk2")
        wv2 = persist.tile([128, 2, CT, HPC, DH], bf16, tag="wv2")
        qT = persist.tile([128, NP, T], bf16, tag="qT")
        kT = persist.tile([128, NP, T], bf16, tag="kT")
        vsb = persist.tile([128, TK, HPC, DH + 1], bf16, tag="vsb")
        trilT = persist.tile([128, 128], bf16, tag="trilT")

        # constants (outside the timing loop)
        make_upper_triangular(nc, trilT[:, :], val=1.0, diag=True)
        nc.gpsimd.memset(vsb[:, :, :, :], 1.0)

        def mk_qk_halves(xT, wqk, dst, dsti, j, c):
            st = {}

            def h1():
                psw = pssp.tile([128, 1024], f32, tag="pss", name="psw")
                ps = psw[:, 0:512]
                st["ps"] = ps
                for ct in range(4):
                    nc.tensor.matmul(
                        ps[:, :], wqk[:, ct, dsti, j, :],
                        xT[:, ct, bass.ts(c, 512)],
                        start=(ct == 0), stop=False,
                    )

            def h2():
                ps = st["ps"]
                for rep in range(2 if "w" in phases else 1):
                    for ct in range(4 if rep == 0 else 0, CT):
                        nc.tensor.matmul(
                            ps[:, :], wqk[:, ct, dsti, j, :],
                            xT[:, ct, bass.ts(c, 512)],
                            start=False, stop=(ct == CT - 1),
                        )
                nc.vector.tensor_copy(dst[:, j, bass.ts(c, 512)], ps[:, :])

            return (h1, h2)

        def emit_dmas(b):
            # input DMAs, split per contraction tile for early compute start
            for ct in range(CT):
                nc.sync.dma_start(wqk2[:, b, ct], wqk_d[:, ct])
                nc.sync.dma_start(xT2[:, b, ct, :], xT_d[:, ct, :])
                nc.sync.dma_start(wv2[:, b, ct], wv_d[:, ct])

        def body(bi=0, dma_bis=(0,), carry_in=None, pairs_done=0):
            xT = xT2[:, bi]
            wqk = wqk2[:, bi]
            wv = wv2[:, bi]
            for b in dma_bis:
                emit_dmas(b)

            # ---- projection work units (split into halves for fine-grained
            # interleaving into the attention stream) ----
            def qk_halves(dst, dsti, j, c):
                return mk_qk_halves(xT, wqk, dst, dsti, j, c)

            def v_halves(tt):
                st = {}

                def h1():
                    psw = pssp.tile([128, 1024], f32, tag="pss", name="psw")
                    ps = psw[:, 0:512]
                    st["ps"] = ps
                    for ct in range(4):
                        nc.tensor.matmul(
                            ps[:, :], xT[:, ct, bass.ts(tt, 128)], wv[:, ct, :, :],
                            start=(ct == 0), stop=False,
                        )

                def h2():
                    ps = st["ps"]
                    for ct in range(4, CT):
                        nc.tensor.matmul(
                            ps[:, :], xT[:, ct, bass.ts(tt, 128)], wv[:, ct, :, :],
                            start=False, stop=(ct == CT - 1),
                        )
                    nc.vector.tensor_copy(
                        vsb[:, tt, :, 0:DH],
                        ps[:, :].rearrange("p (h d) -> p h d", h=HPC),
                    )

                return (h1, h2)

            def emit_qk_unit(dst, dsti, j, c):
                h1, h2 = qk_halves(dst, dsti, j, c)
                h1()
                h2()

            def emit_v_unit(tt):
                h1, h2 = v_halves(tt)
                h1()
                h2()

            if "2" not in phases:
                for j in range(NP):
                    for c in range(TC):
                        emit_qk_unit(qT, 0, j, c)
                        emit_qk_unit(kT, 1, j, c)
                for tt in range(TK):
                    emit_v_unit(tt)
                # DCE-proof consumer: write a sliver of the projections out
                nc.gpsimd.dma_start(out_d[0, 0:8, :].rearrange("a b -> (a b)"),
                                    qT[0:1, 0, 0:512])
                nc.gpsimd.dma_start(out_d[1, 0:8, :].rearrange("a b -> (a b)"),
                                    kT[0:1, 0, 0:512])
                nc.gpsimd.dma_start(out_d[2, 0:8, :].rearrange("a b -> (a b)"),
                                    vsb[0:1, 0, :, :].rearrange("p h d -> (p h d)")[0:512])
                return

            # queues of deferred work-unit halves, drained inside attention
            # chunks. vq = must-finish-this-chunk (v units); inject_q = any
            # time before the owning pair's attention starts.
            inject_q = []
            vq = []
            open_h2 = [None]  # second half of a popped unit, emitted next

            def inject(n):
                for _ in range(n):
                    if open_h2[0] is not None:
                        h2, open_h2[0] = open_h2[0], None
                        h2()
                        continue
                    src = vq if vq else inject_q
                    if not src:
                        return
                    h1, h2 = src.pop(0)
                    h1()
                    open_h2[0] = h2

            def drain_open():
                if open_h2[0] is not None:
                    h2, open_h2[0] = open_h2[0], None
                    h2()

            def drain_vq():
                drain_open()
                while vq:
                    h1, h2 = vq.pop(0)
                    h1()
                    h2()

            if "noinj" in variant:
                for j in range(NP):
                    for c in range(TC):
                        emit_qk_unit(qT, 0, j, c)
                        emit_qk_unit(kT, 1, j, c)
                for tt in range(TK):
                    emit_v_unit(tt)

            # ---- attention, pipelined across head pairs ----
            for j in range(NP):
                hA, hB = 2 * j, 2 * j + 1
                # leftovers belong to pair j: must be emitted before its attn
                drain_open()
                inject(len(inject_q))
                if j + 1 < NP and j + 1 >= pairs_done and "noinj" not in variant:
                    nxt = j + 1
                    for cc in range(TC):
                        inject_q.append(qk_halves(qT, 0, nxt, cc))
                        inject_q.append(qk_halves(kT, 1, nxt, cc))
                elif j + 1 == NP and carry_in:
                    # next body's pair-0 projections fill this ACT-paced pair
                    inject_q.extend(carry_in)

                for c in range(TC):
                    if j == 0 and "noinj" not in variant:
                        drain_open()
                        if c == 0 and pairs_done == 0:
                            emit_qk_unit(qT, 0, 0, 0)
                            emit_qk_unit(kT, 1, 0, 0)
                        if c + 1 < TC and pairs_done == 0:
                            # next chunk's q/k ride the must-drain queue so
                            # chunk c+1 starts with its S matmuls immediately
                            vq.append(qk_halves(qT, 0, 0, c + 1))
                            vq.append(qk_halves(kT, 1, 0, c + 1))
                        vq.extend(v_halves(tt) for tt in range(4 * c, 4 * c + 4))
                    nr = 4 * c + 4
                    poA = pop.tile([128, 512], f32, tag="po")
                    poB = pop.tile([128, 512], f32, tag="po")
                    pss = {}
                    pts = {}

                    def emit_S(r):
                        diag = r >= 4 * c
                        vlo = max(0, r - 4 * c) * 128
                        ps = pssp.tile([128, 1024], f32, tag="pss")
                        pss[r] = ps
                        for rep in range(2 if "y" in phases else 1):
                            nc.tensor.matmul(
                                ps[:, vlo:512], kT[0:64, j, bass.ts(r, 128)],
                                qT[0:64, j, 512 * c + vlo:512 * (c + 1)],
                                start=True, stop=True, tile_position=(0, 0),
                            )
                            # head B shifted left by vlo so the two heads'
                            # valid regions are adjacent -> one gap-free exp
                            nc.tensor.matmul(
                                ps[:, 512:1024 - vlo], kT[64:128, j, bass.ts(r, 128)],
                                qT[64:128, j, 512 * c + vlo:512 * (c + 1)],
                                start=True, stop=True, tile_position=(64, 0),
                            )
                        pt = ptp.tile([128, 1024], bf16, tag="pt")
                        pts[r] = pt
                        for rep in range(2 if "x" in phases else 1):
                            nc.scalar.activation(pt[:, vlo:1024 - vlo],
                                                 ps[:, vlo:1024 - vlo],
                                                 EXP, scale=0.125)
                        if diag:  # diagonal tile: tril mask on DVE
                            nc.vector.tensor_mul(
                                pt[:, vlo:vlo + 128], pt[:, vlo:vlo + 128],
                                trilT[:, :])
                            nc.vector.tensor_mul(
                                pt[:, 512:640], pt[:, 512:640],
                                trilT[:, :])

                    def emit_PV(r):
                        if r >= 4 * c:
                            drain_vq()  # vsb[r] must be written before use
                        vlo = max(0, (r - 4 * c)) * 128
                        pt = pts.pop(r)
                        pss.pop(r)
                        for rep in range(2 if "z" in phases else 1):
                            # rep 1 re-accumulates: doubles out AND rowsum, so
                            # the normalized output is unchanged.
                            nc.tensor.matmul(
                                poA[0:DH + 1, vlo:512], vsb[:, r, hA, :],
                                pt[:, vlo:512],
                                start=(r == 0 and rep == 0), stop=(r == nr - 1),
                                skip_group_check=True,
                            )
                            nc.tensor.matmul(
                                poB[0:DH + 1, vlo:512], vsb[:, r, hB, :],
                                pt[:, 512:1024 - vlo],
                                start=(r == 0 and rep == 0), stop=(r == nr - 1),
                                skip_group_check=True,
                            )

                    for rr_i in range(0, nr, 2):
                        emit_S(rr_i)
                        emit_S(rr_i + 1)
                        if "i2" not in variant:
                            inject(1)
                        if rr_i >= 2:
                            emit_PV(rr_i - 2)
                            emit_PV(rr_i - 1)
                            inject(2 if "i2" in variant else 1)
                    emit_PV(nr - 2)
                    emit_PV(nr - 1)
                    inject(2 if "i2" in variant else 1)

                    # normalize + write natural-layout output rows for this chunk
                    drain_open()
                    for h, po in ((hA, poA), (hB, poB)):
                        ou_s = normp.tile([DH + 16, 512], bf16, tag="ou_s")
                        nc.vector.tensor_copy(ou_s[0:DH + 1, :], po[0:DH + 1, :])
                        # xbar transpose [80,512] -> [512,80]; row t lands at
                        # (p, x) = (t%128, t//128), i.e. t = 128x + p
                        on_T = normp.tile([128, 4, DH + 16], bf16, tag="on_T")
                        nc.sync.dma_start_transpose(on_T[:, :, :], ou_s[:, :])
                        rsc = normp.tile([128, 4], f32, tag="rsc")
                        nc.vector.reciprocal(rsc[:, :], on_T[:, :, DH])
                        on_t = normp.tile([128, 4, DH], bf16, tag="on_t")
                        for t4 in range(4):
                            nc.vector.tensor_scalar_mul(
                                on_t[:, t4, :], on_T[:, t4, 0:DH],
                                rsc[:, t4:t4 + 1])
                        nc.gpsimd.dma_start(
                            out_d[h, bass.ts(c, 512), :].rearrange(
                                "(x p) d -> p x d", p=128),
                            on_t[:, :, :])

            # all queued work (incl. next body's carry) must emit in this body
            drain_open()
            inject(len(inject_q))

        if loop_n > 1:
            nbody = 4 if "quad" in variant else (2 if "dbl" in variant else 1)
            with tc.For_i(0, loop_n, 1):
                for b_i in range(nbody):
                    bi = b_i % 2
                    if b_i + 1 < nbody:
                        nxt = (b_i + 1) % 2
                        carry = []
                        for jj in range(2):
                            for cc in range(TC):
                                carry.append(mk_qk_halves(
                                    xT2[:, nxt], wqk2[:, nxt], qT, 0, jj, cc))
                                carry.append(mk_qk_halves(
                                    xT2[:, nxt], wqk2[:, nxt], kT, 1, jj, cc))
                    else:
                        carry = None
                    body(bi, dma_bis=((0, 1) if b_i == 0 else ()),
                         carry_in=carry, pairs_done=(2 if b_i > 0 else 0))
        else:
            body()

    nc.compile()
    return nc


def _prep_core_inputs(x, Wq, Wk, Wv, core):
    bf = ml_dtypes.bfloat16
    b = core // 2
    hs = (core % 2) * HPC
    # x^T in SBUF layout [p, ct, t]
    xT = np.ascontiguousarray(x[b].T).astype(bf)          # [C, T]
    xT = xT.reshape(CT, 128, T).transpose(1, 0, 2)        # [128, CT, T]
    # q/k weights packed per head pair: [128, CT, 2, NP, 128]
    wqk = np.empty((128, CT, 2, NP, 128), dtype=bf)
    for dsti, W in ((0, Wq), (1, Wk)):
        for j in range(NP):
            wpair = np.concatenate(
                [W[hs + 2 * j], W[hs + 2 * j + 1]], axis=1)  # [C, 128]
            wqk[:, :, dsti, j, :] = (
                wpair.reshape(CT, 128, 128).transpose(1, 0, 2).astype(bf))
    # v weights natural: [128, CT, HPC, DH]
    wv = Wv[hs:hs + HPC].transpose(1, 0, 2)               # [C, HPC, DH]
    wv = wv.reshape(CT, 128, HPC, DH).transpose(1, 0, 2, 3).astype(bf)
    return {
        "xT": np.ascontiguousarray(xT),
        "wqk": np.ascontiguousarray(wqk),
        "wv": np.ascontiguousarray(wv),
    }


def run_on_device(inputs, loop_n=1, trace=False, phases="123"):
    """Build (cached), run on 8 cores, return per-core results."""
    from concourse.bass_utils import run_bass_kernel_spmd

    import os
    variant = os.environ.get("KVAR", "")
    key = (loop_n, phases, variant)
    if key not in _cache:
        _cache[key] = build_program(loop_n, phases, variant)
    nc = _cache[key]
    in_maps = [
        _prep_core_inputs(inputs["x"], inputs["Wq"], inputs["Wk"], inputs["Wv"], c)
        for c in range(NCORES)
    ]
    res = run_bass_kernel_spmd(nc, in_maps, list(range(NCORES)), trace=trace)
    return res


def kernel(x, Wq, Wk, Wv):
    res = run_on_device({"x": x, "Wq": Wq, "Wk": Wk, "Wv": Wv})
    out = np.empty((B, T, H * DH), np.float32)
    for core in range(NCORES):
        b = core // 2
        hs = (core % 2) * HPC
        o = np.asarray(res.results[core]["out"], dtype=np.float32)  # [HPC, T, DH]
        out[b, :, hs * DH:(hs + HPC) * DH] = o.transpose(1, 0, 2).reshape(T, HPC * DH)
    return out


# revision 52
# speedup vs baseline: 1.0802x; 1.0401x over previous
"""Multi-head causal attention (B=4, T=2048, C=1024, H=16, DH=64) on 8 trn2 cores.

Sharding: core c owns batch b = c//2 and heads [8*(c%2), 8*(c%2)+8)  (DP over B x TP over H).

Per-core device kernel (all matmuls bf16, fp32 accumulate), software-pipelined
so ACT(exp) of head-pair j overlaps PE work of pair j+1:
  - q^T/k^T projections: head-pair-packed weights [128c, 128(2x64d)] -> one
    matmul per (pair, chunk, ct), M=128.
  - v: natural layout [t, (h d)], heads packed in N=512.
  - attention per (pair, q-chunk 512): loop causal k-tiles:
    S^T = k q^T (row-tiled pair, K=64x2, N trimmed to 512-vlo on diagonal
    tiles) -> exp on ACT (scale=1/8, masked regions skipped) -> bf16 P^T
    -> diag tril mask -> out^T[65, 512] += v_aug.T @ P^T (row 64 = rowsum).
  - normalize: bf16 PE-transposes to partition-major, reciprocal, broadcast
    multiply, write out^T bf16.
Projection/v units for the NEXT pair are interleaved between PV groups so the
PE never stalls waiting for ACT.
Host: transposes x / packs weights into SBUF-ready layouts (bf16), transposes
per-head out^T back into [B, T, H*DH] and casts to f32.
"""

import numpy as np
import ml_dtypes

B, T, C, H, DH = 4, 2048, 1024, 16, 64
NCORES = 8
HPC = H // 2  # 8 heads per core
NP = HPC // 2  # 4 head pairs per core
CT = C // 128  # 8 contraction tiles
TC = T // 512  # 4 q-chunks
TK = T // 128  # 16 k-tiles

_cache = {}


def build_program(loop_n=1, phases="123", variant=""):
    import concourse.bass as bass
    import concourse.bacc as bacc
    import concourse.mybir as mybir
    import concourse.tile as tile
    from concourse.masks import (make_upper_triangular, make_lower_triangular,
                                 make_identity)
    from contextlib import ExitStack

    f32 = mybir.dt.float32
    bf16 = mybir.dt.bfloat16
    EXP = mybir.ActivationFunctionType.Exp

    nc = bacc.Bacc("TRN2", target_bir_lowering=False, debug=False, num_devices=NCORES)
    xT_d = nc.dram_tensor("xT", [128, CT, T], bf16, kind="ExternalInput")
    wqk_d = nc.dram_tensor("wqk", [128, CT, 2, NP, 128], bf16, kind="ExternalInput")
    wv_d = nc.dram_tensor("wv", [128, CT, HPC, DH], bf16, kind="ExternalInput")
    out_d = nc.dram_tensor("out", [HPC, T, DH], bf16, kind="ExternalOutput")

    with tile.TileContext(nc) as tc, ExitStack() as ctx:
        persist = ctx.enter_context(tc.tile_pool(name="persist", bufs=1))
        ptp = ctx.enter_context(tc.tile_pool(name="ptp", bufs=6))
        normp = ctx.enter_context(tc.tile_pool(name="normp", bufs=4))
        # one shared rotation for S-score tiles AND projection accumulators
        pssp = ctx.enter_context(tc.tile_pool(name="pssp", bufs=3, space="PSUM"))
        pop = ctx.enter_context(tc.tile_pool(name="pop", bufs=2, space="PSUM"))

        # persistent SBUF; inputs double-buffered so the next loop body's
        # DMAs overlap this body's compute
        xT2 = persist.tile([128, 2, CT, T], bf16, tag="xT2")
        wqk2 = persist.tile([128, 2, CT, 2, NP, 128], bf16, tag="wqk2")
        wv2 = persist.tile([128, 2, CT, HPC, DH], bf16, tag="wv2")
        qT = persist.tile([128, NP, T], bf16, tag="qT")
        kT = persist.tile([128, NP, T], bf16, tag="kT")
        vsb = persist.tile([128, TK, HPC, DH + 1], bf16, tag="vsb")
        trilT = persist.tile([128, 128], bf16, tag="trilT")

        # constants (outside the timing loop)
        make_upper_triangular(nc, trilT[:, :], val=1.0, diag=True)
        nc.gpsimd.memset(vsb[:, :, :, :], 1.0)

        def mk_qk_halves(xT, wqk, dst, dsti, j, c):
            st = {}

            def h1():
                psw = pssp.tile([128, 1024], f32, tag="pss", name="psw")
                ps = psw[:, 0:512]
                st["ps"] = ps
                for ct in range(4):
                    nc.tensor.matmul(
                        ps[:, :], wqk[:, ct, dsti, j, :],
                        xT[:, ct, bass.ts(c, 512)],
                        start=(ct == 0), stop=False,
                    )

            def h2():
                ps = st["ps"]
                for rep in range(2 if "w" in phases else 1):
                    for ct in range(4 if rep == 0 else 0, CT):
                        nc.tensor.matmul(
                            ps[:, :], wqk[:, ct, dsti, j, :],
                            xT[:, ct, bass.ts(c, 512)],
                            start=False, stop=(ct == CT - 1),
                        )
                nc.vector.tensor_copy(dst[:, j, bass.ts(c, 512)], ps[:, :])

            return (h1, h2)

        def emit_dmas(b):
            # input DMAs, split per contraction tile for early compute start
            for ct in range(CT):
                nc.sync.dma_start(wqk2[:, b, ct], wqk_d[:, ct])
                nc.sync.dma_start(xT2[:, b, ct, :], xT_d[:, ct, :])
                nc.sync.dma_start(wv2[:, b, ct], wv_d[:, ct])

        def body(bi=0, dma_bis=(0,), carry_in=None, pairs_done=0):
            xT = xT2[:, bi]
            wqk = wqk2[:, bi]
            wv = wv2[:, bi]
            for b in dma_bis:
                emit_dmas(b)

            # ---- projection work units (split into halves for fine-grained
            # interleaving into the attention stream) ----
            def qk_halves(dst, dsti, j, c):
                return mk_qk_halves(xT, wqk, dst, dsti, j, c)

            def v_halves(tt):
                st = {}

                def h1():
                    psw = pssp.tile([128, 1024], f32, tag="pss", name="psw")
                    ps = psw[:, 0:512]
                    st["ps"] = ps
                    for ct in range(4):
                        nc.tensor.matmul(
                            ps[:, :], xT[:, ct, bass.ts(tt, 128)], wv[:, ct, :, :],
                            start=(ct == 0), stop=False,
                        )

                def h2():
                    ps = st["ps"]
                    for ct in range(4, CT):
                        nc.tensor.matmul(
                            ps[:, :], xT[:, ct, bass.ts(tt, 128)], wv[:, ct, :, :],
                            start=False, stop=(ct == CT - 1),
                        )
                    nc.vector.tensor_copy(
                        vsb[:, tt, :, 0:DH],
                        ps[:, :].rearrange("p (h d) -> p h d", h=HPC),
                    )

                return (h1, h2)

            def emit_qk_unit(dst, dsti, j, c):
                h1, h2 = qk_halves(dst, dsti, j, c)
                h1()
                h2()

            def emit_v_unit(tt):
                h1, h2 = v_halves(tt)
                h1()
                h2()

            if "2" not in phases:
                for j in range(NP):
                    for c in range(TC):
                        emit_qk_unit(qT, 0, j, c)
                        emit_qk_unit(kT, 1, j, c)
                for tt in range(TK):
                    emit_v_unit(tt)
                # DCE-proof consumer: write a sliver of the projections out
                nc.gpsimd.dma_start(out_d[0, 0:8, :].rearrange("a b -> (a b)"),
                                    qT[0:1, 0, 0:512])
                nc.gpsimd.dma_start(out_d[1, 0:8, :].rearrange("a b -> (a b)"),
                                    kT[0:1, 0, 0:512])
                nc.gpsimd.dma_start(out_d[2, 0:8, :].rearrange("a b -> (a b)"),
                                    vsb[0:1, 0, :, :].rearrange("p h d -> (p h d)")[0:512])
                return

            # queues of deferred work-unit halves, drained inside attention
            # chunks. vq = must-finish-this-chunk (v units); inject_q = any
            # time before the owning pair's attention starts.
            inject_q = []
            vq = []
            open_h2 = [None]  # second half of a popped unit, emitted next

            def inject(n):
                for _ in range(n):
                    if open_h2[0] is not None:
                        h2, open_h2[0] = open_h2[0], None
                        h2()
                        continue
                    src = vq if vq else inject_q
                    if not src:
                        return
                    h1, h2 = src.pop(0)
                    h1()
                    open_h2[0] = h2

            def drain_open():
                if open_h2[0] is not None:
                    h2, open_h2[0] = open_h2[0], None
                    h2()

            def drain_vq():
                drain_open()
                while vq:
                    h1, h2 = vq.pop(0)
                    h1()
                    h2()

            if "noinj" in variant:
                for j in range(NP):
                    for c in range(TC):
                        emit_qk_unit(qT, 0, j, c)
                        emit_qk_unit(kT, 1, j, c)
                for tt in range(TK):
                    emit_v_unit(tt)

            # ---- attention, pipelined across head pairs ----
            for j in range(NP):
                hA, hB = 2 * j, 2 * j + 1
                # leftovers belong to pair j: must be emitted before its attn
                drain_open()
                inject(len(inject_q))
                if j + 1 < NP and j + 1 >= pairs_done and "noinj" not in variant:
                    nxt = j + 1
                    for cc in range(TC):
                        inject_q.append(qk_halves(qT, 0, nxt, cc))
                        inject_q.append(qk_halves(kT, 1, nxt, cc))
                elif j + 1 == NP and carry_in:
                    # next body's pair-0 projections fill this ACT-paced pair
                    inject_q.extend(carry_in)

                for c in range(TC):
                    if j == 0 and "noinj" not in variant:
                        drain_open()
                        if c == 0 and pairs_done == 0:
                            emit_qk_unit(qT, 0, 0, 0)
                            emit_qk_unit(kT, 1, 0, 0)
                        if c + 1 < TC and pairs_done == 0:
                            # next chunk's q/k ride the must-drain queue so
                            # chunk c+1 starts with its S matmuls immediately
                            vq.append(qk_halves(qT, 0, 0, c + 1))
                            vq.append(qk_halves(kT, 1, 0, c + 1))
                        vq.extend(v_halves(tt) for tt in range(4 * c, 4 * c + 4))
                    nr = 4 * c + 4
                    poA = pop.tile([128, 512], f32, tag="po")
                    poB = pop.tile([128, 512], f32, tag="po")
                    pss = {}
                    pts = {}

                    def emit_S(r):
                        diag = r >= 4 * c
                        vlo = max(0, r - 4 * c) * 128
                        ps = pssp.tile([128, 1024], f32, tag="pss")
                        pss[r] = ps
                        for rep in range(2 if "y" in phases else 1):
                            nc.tensor.matmul(
                                ps[:, vlo:512], kT[0:64, j, bass.ts(r, 128)],
                                qT[0:64, j, 512 * c + vlo:512 * (c + 1)],
                                start=True, stop=True, tile_position=(0, 0),
                            )
                            # head B shifted left by vlo so the two heads'
                            # valid regions are adjacent -> one gap-free exp
                            nc.tensor.matmul(
                                ps[:, 512:1024 - vlo], kT[64:128, j, bass.ts(r, 128)],
                                qT[64:128, j, 512 * c + vlo:512 * (c + 1)],
                                start=True, stop=True, tile_position=(64, 0),
                            )
                        pt = ptp.tile([128, 1024], bf16, tag="pt")
                        pts[r] = pt
                        for rep in range(2 if "x" in phases else 1):
                            nc.scalar.activation(pt[:, vlo:1024 - vlo],
                                                 ps[:, vlo:1024 - vlo],
                                                 EXP, scale=0.125)
                        if diag:  # diagonal tile: tril mask on DVE
                            nc.vector.tensor_mul(
                                pt[:, vlo:vlo + 128], pt[:, vlo:vlo + 128],
                                trilT[:, :])
                            nc.vector.tensor_mul(
                                pt[:, 512:640], pt[:, 512:640],
                                trilT[:, :])

                    def emit_PV(r):
                        if r >= 4 * c:
                            drain_vq()  # vsb[r] must be written before use
                        vlo = max(0, (r - 4 * c)) * 128
                        pt = pts.pop(r)
                        pss.pop(r)
                        for rep in range(2 if "z" in phases else 1):
                            # rep 1 re-accumulates: doubles out AND rowsum, so
                            # the normalized output is unchanged.
                            nc.tensor.matmul(
                                poA[0:DH + 1, vlo:512], vsb[:, r, hA, :],
                                pt[:, vlo:512],
                                start=(r == 0 and rep == 0), stop=(r == nr - 1),
                                skip_group_check=True,
                            )
                            nc.tensor.matmul(
                                poB[0:DH + 1, vlo:512], vsb[:, r, hB, :],
                                pt[:, 512:1024 - vlo],
                                start=(r == 0 and rep == 0), stop=(r == nr - 1),
                                skip_group_check=True,
                            )

                    for rr_i in range(0, nr, 2):
                        emit_S(rr_i)
                        emit_S(rr_i + 1)
                        if "i2" not in variant:
                            inject(1)
                        if rr_i >= 2:
                            emit_PV(rr_i - 2)
                            emit_PV(rr_i - 1)
                            inject(2 if "i2" in variant else 1)
                    emit_PV(nr - 2)
                    emit_PV(nr - 1)
                    inject(2 if "i2" in variant else 1)

                    # normalize + write natural-layout output rows for this chunk
                    drain_open()
                    for h, po in ((hA, poA), (hB, poB)):
                        ou_s = normp.tile([DH + 16, 512], bf16, tag="ou_s")
                        nc.vector.tensor_copy(ou_s[0:DH + 1, :], po[0:DH + 1, :])
                        # xbar transpose [80,512] -> [512,80]; row t lands at
                        # (p, x) = (t%128, t//128), i.e. t = 128x + p
                        on_T = normp.tile([128, 4, DH + 16], bf16, tag="on_T")
                        nc.sync.dma_start_transpose(on_T[:, :, :], ou_s[:, :])
                        rsc = normp.tile([128, 4], f32, tag="rsc")
                        nc.vector.reciprocal(rsc[:, :], on_T[:, :, DH])
                        on_t = normp.tile([128, 4, DH], bf16, tag="on_t")
                        for t4 in range(4):
                            nc.vector.tensor_scalar_mul(
                                on_t[:, t4, :], on_T[:, t4, 0:DH],
                                rsc[:, t4:t4 + 1])
                        nc.gpsimd.dma_start(
                            out_d[h, bass.ts(c, 512), :].rearrange(
                                "(x p) d -> p x d", p=128),
                            on_t[:, :, :])

            # all queued work (incl. next body's carry) must emit in this body
            drain_open()
            inject(len(inject_q))

        def mk_carry(nxt):
            carry = []
            for jj in range(2):
                for cc in range(TC):
                    carry.append(mk_qk_halves(
                        xT2[:, nxt], wqk2[:, nxt], qT, 0, jj, cc))
                    carry.append(mk_qk_halves(
                        xT2[:, nxt], wqk2[:, nxt], kT, 1, jj, cc))
            return carry

        if loop_n > 1 and "dbl" in variant:
            # steady-state wrap-around: each body prefetches the OTHER
            # buffer's inputs and projects the other body's pairs 0-1 during
            # its own ACT-paced final pair. Prologue primes the first body.
            emit_dmas(0)
            for h1, h2 in mk_carry(0):
                h1()
                h2()
            with tc.For_i(0, loop_n, 1):
                body(0, dma_bis=(1,), carry_in=mk_carry(1), pairs_done=2)
                body(1, dma_bis=(0,), carry_in=mk_carry(0), pairs_done=2)
        elif loop_n > 1:
            nbody = 4 if "quad" in variant else 1
            with tc.For_i(0, loop_n, 1):
                for b_i in range(nbody):
                    bi = b_i % 2
                    carry = mk_carry((b_i + 1) % 2) if b_i + 1 < nbody else None
                    body(bi, dma_bis=((0, 1) if b_i == 0 else ()),
                         carry_in=carry, pairs_done=(2 if b_i > 0 else 0))
        else:
            body()

    nc.compile()
    return nc


def _prep_core_inputs(x, Wq, Wk, Wv, core):
    bf = ml_dtypes.bfloat16
    b = core // 2
    hs = (core % 2) * HPC
    # x^T in SBUF layout [p, ct, t]
    xT = np.ascontiguousarray(x[b].T).astype(bf)          # [C, T]
    xT = xT.reshape(CT, 128, T).transpose(1, 0, 2)        # [128, CT, T]
    # q/k weights packed per head pair: [128, CT, 2, NP, 128]
    wqk = np.empty((128, CT, 2, NP, 128), dtype=bf)
    for dsti, W in ((0, Wq), (1, Wk)):
        for j in range(NP):
            wpair = np.concatenate(
                [W[hs + 2 * j], W[hs + 2 * j + 1]], axis=1)  # [C, 128]
            wqk[:, :, dsti, j, :] = (
                wpair.reshape(CT, 128, 128).transpose(1, 0, 2).astype(bf))
    # v weights natural: [128, CT, HPC, DH]
    wv = Wv[hs:hs + HPC].transpose(1, 0, 2)               # [C, HPC, DH]
    wv = wv.reshape(CT, 128, HPC, DH).transpose(1, 0, 2, 3).astype(bf)
    return {
        "xT": np.ascontiguousarray(xT),
        "wqk": np.ascontiguousarray(wqk),
        "wv": np.ascontiguousarray(wv),
    }


def run_on_device(inputs, loop_n=1, trace=False, phases="123"):
    """Build (cached), run on 8 cores, return per-core results."""
    from concourse.bass_utils import run_bass_kernel_spmd

    import os
    variant = os.environ.get("KVAR", "")
    key = (loop_n, phases, variant)
    if key not in _cache:
        _cache[key] = build_program(loop_n, phases, variant)
    nc = _cache[key]
    in_maps = [
        _prep_core_inputs(inputs["x"], inputs["Wq"], inputs["Wk"], inputs["Wv"], c)
        for c in range(NCORES)
    ]
    res = run_bass_kernel_spmd(nc, in_maps, list(range(NCORES)), trace=trace)
    return res


def kernel(x, Wq, Wk, Wv):
    res = run_on_device({"x": x, "Wq": Wq, "Wk": Wk, "Wv": Wv})
    out = np.empty((B, T, H * DH), np.float32)
    for core in range(NCORES):
        b = core // 2
        hs = (core % 2) * HPC
        o = np.asarray(res.results[core]["out"], dtype=np.float32)  # [HPC, T, DH]
        out[b, :, hs * DH:(hs + HPC) * DH] = o.transpose(1, 0, 2).reshape(T, HPC * DH)
    return out
